# revision 13
# baseline (speedup 1.0000x reference)
"""CRF-RNN mean-field iteration kernel for Trainium2 (8 NeuronCores).

Math (per batch b, NITERS=5):
    D_norm = W / W.sum(axis=1, keepdims)          # row-normalized affinity [n, n]
    qVals  = uniqs = seg.reshape(d, n)
    loop:  Q = softmax(qVals, axis=0)             # over class dim d=21
           seg_diff   = Q @ D_norm^T              # [d, n]
           seg_update = weights @ seg_diff
           qVals      = uniqs - seg_update

Sharding: batch b -> core pair (2b, 2b+1); each core owns half the output
positions (m rows of W). The contraction runs over all n, so the host feeds
each core its W block PRE-TRANSPOSED (pure layout prep, like the other
host-side input permutations): the device streams W^T slabs from HBM once
(~32MB/core, the DMA roofline), casts them to fp8-e4m3 on the Scalar/Vector
engines (split so neither gates the DMA cadence), and keeps W^T resident in
SBUF across all 5 iterations. The main matmuls run in fp8 DoubleRow mode
(256-wide contraction per pass). Row-normalization comes for free: iteration
0's stationary operand carries an extra all-ones column, so PSUM row 21
accumulates the W row-sums; the `ut` transpose-matmul (against weights^T
padded to 22x22 with a unit diagonal tail) lands them position-major, and a
tiny reciprocal feeds the per-partition scaling of seg_update. Iteration 0's
matmuls interleave with the (DMA-bound) prepass per slab pair. Per iteration
the pair exchanges its half of softmax(Q) via two chunked pairwise
AllGathers; sends fire mid-iteration as each chunk's softmax completes, and
the recv DMAs are emitted by the CONSUMING iteration so they never block a
later send on the in-order sync ring. Iteration boundaries blend: the next
iteration's own-half matmuls are gated per 4-tile group on the producing
tail, keeping the PE stream dense. The instruction stream is identical on
all cores (SPMD): all own/partner asymmetry lives in host-side input
permutations and a select-mask input.
"""

import os
import sys

for _p in ("/opt/trn_rl_repo",):
    if _p not in sys.path:
        sys.path.insert(0, _p)

import numpy as np

BS, D, RC = 4, 21, 64
DE = D + 1        # class dim + rowsum carrier row
N = RC * RC       # 4096 positions
NH = N // 2       # 2048 positions per core (own half)
NT = 32           # 128-wide position tiles (global)
NTO = 16          # own tiles
NT2 = 16          # 256-wide fp8 pair tiles (global)
NSLAB = 32        # W^T n-slabs of 128 rows
QPAD = 32         # class-dim padding for fp8 DoubleRow lhsT stride
CHUNK = NTO // 2  # own tiles per exchange chunk
NITERS = int(os.environ.get("CRF_NITERS", "5"))
NCORES = 8
RG = [[0, 1], [2, 3], [4, 5], [6, 7]]

LAST_EXEC_NS = None
_CACHE = {}


def _install_ntff_hook():
    """Best-effort registration of the axon NTFF profile hook (image antenv
    lacks axon_hooks, so trn_boot could not register it)."""
    try:
        import types

        if "antenv.axon_hooks" in sys.modules:
            return
        holder = [None]
        m = types.ModuleType("antenv.axon_hooks")
        m.set_axon_ntff_profile_hook = lambda h: holder.__setitem__(0, h)
        m.get_axon_ntff_profile_hook = lambda: holder[0]
        sys.modules["antenv.axon_hooks"] = m
        import antenv

        antenv.axon_hooks = m
        from trn_agent_boot.trn_boot import _ntff_profile_via_ctypes

        m.set_axon_ntff_profile_hook(
            _ntff_profile_via_ctypes("/opt/axon/libaxon_pjrt.so")
        )
    except Exception:
        pass


def _build(niters):
    from concourse import bacc, bass, tile, mybir

    fp32, fp16 = mybir.dt.float32, mybir.dt.float16
    sdt = mybir.dt.float8e4
    AF = mybir.ActivationFunctionType
    ALU = mybir.AluOpType
    ntile = NT2
    half = ntile // 2
    perf = mybir.MatmulPerfMode.DoubleRow

    nc = bacc.Bacc(None, target_bir_lowering=False)

    wt_in_hbm = nc.dram_tensor("w", (N, NH), fp32, kind="ExternalInput")
    segt_in = nc.dram_tensor("segt", (128, NT, D), fp32, kind="ExternalInput")
    wm_in = nc.dram_tensor("wt", (DE, DE), fp32, kind="ExternalInput")
    sel_in = nc.dram_tensor("sel", (128, 2), fp32, kind="ExternalInput")
    out_t = nc.dram_tensor("out", (128, NTO, D), fp32, kind="ExternalOutput")

    n_ex = max(0, niters - 1)
    cc_ins = [
        [
            nc.dram_tensor(f"cc_in{k}_{c}", (128, CHUNK * QPAD), sdt, kind="Internal")
            for c in range(2)
        ]
        for k in range(n_ex)
    ]
    cc_outs = [
        [
            nc.dram_tensor(
                f"cc_out{k}_{c}", (2, 128, CHUNK * QPAD), sdt, kind="Internal"
            )
            for c in range(2)
        ]
        for k in range(n_ex)
    ]

    with tile.TileContext(nc) as tc:
        with (
            tc.tile_pool(name="wt_res", bufs=1) as wt_res,
            tc.tile_pool(name="slab32", bufs=4) as slab32p,
            tc.tile_pool(name="state", bufs=1) as state,
            tc.tile_pool(name="qt", bufs=2) as qtp,
            tc.tile_pool(name="work", bufs=2) as work,
            tc.tile_pool(name="ps_mm", bufs=1, space=bass.MemorySpace.PSUM) as ps_mm,
            tc.tile_pool(name="ps_ut", bufs=2, space=bass.MemorySpace.PSUM) as ps_ut,
            tc.tile_pool(name="ps_jk", bufs=1, space=bass.MemorySpace.PSUM) as ps_jk,
        ):
            # ---- small inputs (ACT HWDGE ring; slab DMAs own the SP ring) --
            wm32 = state.tile([DE, DE], fp32)
            nc.scalar.dma_start(wm32[:], wm_in[:])
            wm16 = state.tile([DE, DE], fp16)
            nc.gpsimd.tensor_copy(wm16[:], wm32[:])
            segt = state.tile([128, NT, D], fp32)
            nc.scalar.dma_start(segt[:], segt_in[:])
            selt = state.tile([128, 2], fp32)
            nc.scalar.dma_start(selt[:], sel_in[:])
            zbias = state.tile([128, 1], fp32)
            nc.gpsimd.memset(zbias[:], 0.0)
            # mask for predicated partner select: nonzero where slot1=partner
            selmask = state.tile([128, CHUNK * QPAD], mybir.dt.uint8)
            nc.gpsimd.tensor_scalar_mul(
                selmask[:],
                selt[:, 1:2].broadcast_to((128, CHUNK * QPAD)),
                1.0,
            )

            # ---- initial Q = softmax(uniqs); ones in column D for rowsum ---
            ex0 = state.tile([128, NT, D], fp32)
            nc.scalar.activation(ex0[:], segt[:], AF.Exp, bias=zbias[:])
            ssum0 = state.tile([128, NT], fp32)
            nc.vector.reduce_sum(ssum0[:], ex0[:], axis=mybir.AxisListType.X)
            srecip0 = state.tile([128, NT], fp32)
            nc.vector.reciprocal(srecip0[:], ssum0[:])
            qt_own = qtp.tile([128, NTO, QPAD], sdt, tag="qt_own", name="qt_own0")
            qt_par = qtp.tile([128, NTO, QPAD], sdt, tag="qt_par", name="qt_par0")
            nc.vector.tensor_tensor(
                qt_own[:, :, 0:D],
                ex0[:, 0:NTO, :],
                srecip0[:, 0:NTO, None].broadcast_to((128, NTO, D)),
                ALU.mult,
            )
            nc.vector.tensor_tensor(
                qt_par[:, :, 0:D],
                ex0[:, NTO:NT, :],
                srecip0[:, NTO:NT, None].broadcast_to((128, NTO, D)),
                ALU.mult,
            )
            nc.gpsimd.memset(qt_own[:, :, D : D + 1], 1.0)
            nc.gpsimd.memset(qt_par[:, :, D : D + 1], 1.0)

            # ---- resident W^T (fp8, pair-interleaved for DoubleRow) ------
            # wt_all[p, t2, i, j] = W^T[256*t2 + 128*i + p, j]
            wt_all = state.tile([128, NT2, 2, NH], sdt, name="wt_all")
            rs_recg = [
                state.tile([128, 4], fp32, tag=f"rsrec{g}", name=f"rs_rec{g}")
                for g in range(4)
            ]

            # narrow filler matmuls: keep the PE instruction stream dense
            # through cross-engine waits so the HAM clock gate stays open
            junk = ps_jk.tile([D, 512], fp32, name="junk")

            def fillers(n):
                for f in range(n):
                    nc.tensor.matmul(
                        junk[:, 0:D],
                        wm16[0:D, 0:D],
                        wm16[0:D, 0:D],
                        start=True,
                        stop=True,
                    )

            def lhs_of(t, q_own, q_par, de):
                src = q_own if t < half else q_par
                j2 = t % half
                return src[:, 2 * j2 : 2 * j2 + 2, 0:de]

            def rhs_of(t, mc):
                return wt_all[:, t, :, mc * 512 : (mc + 1) * 512]

            class IterEmitter:
                """Emits one mean-field iteration in dependency-friendly
                pieces so matmuls, evacuations, and the softmax tail
                pipeline across engines (and, for iteration 0, interleave
                with the prepass)."""

                def __init__(self, it, q_own, q_par, last):
                    self.it, self.q_own, self.q_par, self.last = it, q_own, q_par, last
                    self.de = DE if it == 0 else D
                    self.pP = ps_mm.tile([DE, NH], fp32, tag="pp", name=f"pp{it}")
                    self.ps16g = [None] * 4
                    self.pUTg = [None] * 4
                    self.qt_next = None
                    if not last:
                        self.qt_next = qtp.tile(
                            [128, NTO, QPAD], sdt, tag="qt_own", name=f"qt_own{it+1}"
                        )

                def phase(self, mms):
                    for t, mc in mms:
                        nc.tensor.matmul(
                            self.pP[0 : self.de, mc * 512 : (mc + 1) * 512],
                            lhs_of(t, self.q_own, self.q_par, self.de),
                            rhs_of(t, mc),
                            start=(t == 0),
                            stop=(t == ntile - 1),
                            perf_mode=perf,
                        )

                def evac(self, mc, on_act=None):
                    t16 = work.tile(
                        [DE, 512], fp16, tag=f"ps16_{mc}", name=f"ps16_{self.it}_{mc}"
                    )
                    src = self.pP[0 : self.de, mc * 512 : (mc + 1) * 512]
                    if on_act is None:
                        on_act = mc % 2 == 1
                    if on_act:
                        nc.scalar.activation(t16[0 : self.de, :], src, AF.Copy)
                    else:
                        nc.vector.tensor_copy(t16[0 : self.de, :], src)
                    self.ps16g[mc] = t16

                def ut(self, g):
                    de = self.de
                    pu = ps_ut.tile(
                        [128, 4, DE], fp32, tag="put", name=f"pUT{self.it}_{g}"
                    )
                    for jj in range(4):
                        nc.tensor.matmul(
                            pu[:, jj, 0:de],
                            self.ps16g[g][0:de, jj * 128 : (jj + 1) * 128],
                            wm16[0:de, 0:de],
                            start=True,
                            stop=True,
                        )
                    self.pUTg[g] = pu
                    if self.it == 0:
                        # PSUM row D carried the W row-sums through ut
                        nc.vector.reciprocal(rs_recg[g][:], pu[:, :, D])

                def tail(self, g):
                    it, sl = self.it, slice(4 * g, 4 * g + 4)
                    upd = work.tile([128, 4, D], fp32, tag=f"upd{g}", name=f"upd{it}_{g}")
                    nc.vector.tensor_tensor(
                        upd[:],
                        self.pUTg[g][:, :, 0:D],
                        rs_recg[g][:, :, None].broadcast_to((128, 4, D)),
                        ALU.mult,
                    )
                    qv = work.tile([128, 4, D], fp32, tag=f"qv{g}", name=f"qv{it}_{g}")
                    nc.gpsimd.tensor_tensor(qv[:], segt[:, sl, :], upd[:], ALU.subtract)
                    if self.last:
                        nc.sync.dma_start(out_t[:, sl, :], qv[:])
                        return
                    exq = work.tile([128, 4, D], fp32, tag=f"exq{g}", name=f"exq{it}_{g}")
                    nc.scalar.activation(exq[:], qv[:], AF.Exp, bias=zbias[:])
                    ssum = work.tile([128, 4], fp32, tag=f"ssum{g}", name=f"ssum{it}_{g}")
                    nc.vector.reduce_sum(ssum[:], exq[:], axis=mybir.AxisListType.X)
                    srec = work.tile([128, 4], fp32, tag=f"srec{g}", name=f"srec{it}_{g}")
                    nc.vector.reciprocal(srec[:], ssum[:])
                    nc.vector.tensor_tensor(
                        self.qt_next[:, sl, 0:D],
                        exq[:],
                        srec[:, :, None].broadcast_to((128, 4, D)),
                        ALU.mult,
                    )

                def send_chunk(self, c):
                    """Send own tiles [8c, 8c+8) of qt_next to the partner.
                    The recv side is emitted by the NEXT iteration (recv_chunk)
                    so no recv DMA ever queues ahead of a later send on the
                    in-order sync ring (a recv blocks on AllGather completion)."""
                    it = self.it
                    csl = slice(CHUNK * c, CHUNK * (c + 1))
                    nc.sync.dma_start(
                        cc_ins[it][c][:],
                        self.qt_next[:, csl, :].rearrange("p a b -> p (a b)"),
                    )
                    nc.gpsimd.collective_compute(
                        "AllGather",
                        ALU.bypass,
                        replica_groups=RG,
                        ins=[cc_ins[it][c][:].opt()],
                        outs=[cc_outs[it][c][:].opt()],
                    )

                def recv_chunk(self, c):
                    """Land the partner half of exchange chunk c of the
                    PREVIOUS round into this iteration's qt_par."""
                    ex = self.it - 1
                    csl = slice(CHUNK * c, CHUNK * (c + 1))
                    dst = self.q_par[:, csl, :].rearrange("p a b -> p (a b)")
                    g1 = work.tile(
                        [128, CHUNK * QPAD], sdt, tag="g1", name=f"g1_{ex}_{c}"
                    )
                    nc.sync.dma_start(dst, cc_outs[ex][c][0][:])
                    nc.sync.dma_start(g1[:], cc_outs[ex][c][1][:])
                    nc.vector.copy_predicated(dst, selmask[:], g1[:])

            # ---- prepass: stream W^T slabs, cast to fp8, fold in iter 0 ----
            fillers(12)
            em = IterEmitter(0, qt_own, qt_par, last=(niters == 1))
            for s in range(NSLAB):
                t2, i = s // 2, s % 2
                w32 = slab32p.tile([128, NH], fp32, tag="w32", name=f"w32_{s}")
                nc.sync.dma_start(w32[:], wt_in_hbm[s * 128 : (s + 1) * 128, :])
                dst = wt_all[:, t2, i, :]
                if s % 8 in (2, 5, 7):
                    nc.vector.tensor_copy(dst, w32[:])
                else:
                    nc.scalar.activation(dst, w32[:], AF.Copy)
                if i == 1:
                    em.phase([(t2, mc) for mc in range(4)])
                    fillers(14)
            for g in range(4):
                em.evac(g, on_act=(g % 2 == 1))
                em.ut(g)
                em.tail(g)
                if niters > 1:
                    if g == 1:
                        em.send_chunk(0)
                    elif g == 3:
                        em.send_chunk(1)
            if niters > 1:
                qt_own = em.qt_next

            # ---- iterations 1..niters-1 ---------------------------------
            for it in range(1, niters):
                qt_par = qtp.tile(
                    [128, NTO, QPAD], sdt, tag="qt_par", name=f"qt_par{it}"
                )
                em = IterEmitter(it, qt_own, qt_par, last=(it == niters - 1))
                em.recv_chunk(0)
                fillers(8)
                # own tiles, gated per 4-tile group on the producing tail;
                # mc=3 last (its pP chunk is freed latest by the prev iter)
                for g in range(4):
                    em.phase([(t, mc) for t in (2 * g, 2 * g + 1) for mc in range(3)])
                em.phase([(t, 3) for t in range(half)])
                em.recv_chunk(1)
                fillers(34 if it == 1 else 10)
                # partner tiles, first exchange chunk
                em.phase([(t, mc) for t in range(half, half + 4) for mc in range(4)])
                fillers(6)
                # partner tiles, second chunk; per-mc finish -> evac/ut/tail
                for mc in range(4):
                    em.phase([(t, mc) for t in range(half + 4, ntile)])
                    em.evac(mc, on_act=(mc in (1, 2)))
                    em.ut(mc)
                    em.tail(mc)
                    if it < niters - 1:
                        if mc == 1:
                            em.send_chunk(0)
                        elif mc == 3:
                            em.send_chunk(1)
                if it < niters - 1:
                    qt_own = em.qt_next

    nc.compile()
    return nc


def _get_nc(niters):
    if niters not in _CACHE:
        _CACHE[niters] = _build(niters)
    return _CACHE[niters]


def kernel(seg, W, weights):
    global LAST_EXEC_NS
    assert seg.shape == (BS, D, RC, RC) and W.shape == (BS, N, N)
    trace = bool(os.environ.get("BASS_TRACE"))
    if trace:
        _install_ntff_hook()

    from concourse.bass_utils import run_bass_kernel_spmd

    nc = _get_nc(NITERS)

    seg32 = np.ascontiguousarray(seg, dtype=np.float32)
    W32 = np.ascontiguousarray(W, dtype=np.float32)
    # weights^T padded to 22x22 with a unit diagonal tail: through the `ut`
    # transpose-matmul, ps16 row 21 (the accumulated W row-sums) passes into
    # pUT column 21 untouched while columns 0..20 get weights @ seg_diff.
    wm_np = np.zeros((DE, DE), np.float32)
    wm_np[0:D, 0:D] = weights.T
    wm_np[D, D] = 1.0

    in_maps = []
    for c in range(NCORES):
        b, h = c // 2, c % 2
        own = slice(NH * h, NH * h + NH)
        par = slice(NH * (1 - h), NH * (1 - h) + NH)
        Wb = W32[b]
        # host-side layout prep: the device wants W^T (contraction index n on
        # partitions), rows in [own, partner] order to match the Q tiles
        w_np = np.ascontiguousarray(
            np.concatenate([Wb[own, own], Wb[own, par]], axis=1).T
        )
        st = seg32[b].reshape(D, N).T  # [n, d]
        st_perm = np.concatenate([st[own], st[par]], axis=0)
        segt_np = np.ascontiguousarray(
            st_perm.reshape(NT, 128, D).transpose(1, 0, 2)
        )
        sel_np = np.zeros((128, 2), np.float32)
        sel_np[:, 0] = float(h)       # gather slot (1-h) = partner
        sel_np[:, 1] = float(1 - h)
        in_maps.append(
            {"w": w_np, "segt": segt_np, "wt": wm_np, "sel": sel_np}
        )

    res = run_bass_kernel_spmd(
        nc, in_maps, core_ids=list(range(NCORES)), trace=trace
    )
    LAST_EXEC_NS = res.exec_time_ns

    out = np.empty((BS, D, N), np.float32)
    for c in range(NCORES):
        b, h = c // 2, c % 2
        qv = res.results[c]["out"]  # [128, NTO, D]
        block = qv.transpose(2, 1, 0).reshape(D, NH)
        out[b][:, NH * h : NH * h + NH] = block
    return out.reshape(BS, D, RC, RC)


if __name__ == "__main__":
    rng = np.random.default_rng(0)
    seg = rng.standard_normal((BS, D, RC, RC)).astype(np.float32)
    W = rng.random((BS, N, N), dtype=np.float32)
    weights = rng.standard_normal((D, D)).astype(np.float32)
    out = kernel(seg=seg, W=W, weights=weights)
    print("out", out.shape, out.dtype, float(np.abs(out).mean()))


# revision 14
# speedup vs baseline: 1.0307x; 1.0307x over previous
"""CRF-RNN mean-field iteration kernel for Trainium2 (8 NeuronCores).

Math (per batch b, NITERS=5):
    D_norm = W / W.sum(axis=1, keepdims)          # row-normalized affinity [n, n]
    qVals  = uniqs = seg.reshape(d, n)
    loop:  Q = softmax(qVals, axis=0)             # over class dim d=21
           seg_diff   = Q @ D_norm^T              # [d, n]
           seg_update = weights @ seg_diff
           qVals      = uniqs - seg_update

Sharding: batch b -> core pair (2b, 2b+1); each core owns half the output
positions (m rows of W). The contraction runs over all n, so the host feeds
each core its W block PRE-TRANSPOSED (pure layout prep, like the other
host-side input permutations): the device streams W^T slabs from HBM once
(~32MB/core, the DMA roofline), casts them to fp8-e4m3 on the Scalar/Vector
engines (split so neither gates the DMA cadence), and keeps W^T resident in
SBUF across all 5 iterations. The main matmuls run in fp8 DoubleRow mode
(256-wide contraction per pass). Row-normalization comes for free: iteration
0's stationary operand carries an extra all-ones column, so PSUM row 21
accumulates the W row-sums; the `ut` transpose-matmul (against weights^T
padded to 22x22 with a unit diagonal tail) lands them position-major, and a
tiny reciprocal feeds the per-partition scaling of seg_update. Iteration 0's
matmuls interleave with the (DMA-bound) prepass per slab pair. Per iteration
the pair exchanges its half of softmax(Q) via two chunked pairwise
AllGathers; sends fire mid-iteration as each chunk's softmax completes, and
the recv DMAs are emitted by the CONSUMING iteration so they never block a
later send on the in-order sync ring. Iteration boundaries blend: the next
iteration's own-half matmuls are gated per 4-tile group on the producing
tail, keeping the PE stream dense. The instruction stream is identical on
all cores (SPMD): all own/partner asymmetry lives in host-side input
permutations and a select-mask input.
"""

import os
import sys

for _p in ("/opt/trn_rl_repo",):
    if _p not in sys.path:
        sys.path.insert(0, _p)

import numpy as np

BS, D, RC = 4, 21, 64
DE = D + 1        # class dim + rowsum carrier row
N = RC * RC       # 4096 positions
NH = N // 2       # 2048 positions per core (own half)
NT = 32           # 128-wide position tiles (global)
NTO = 16          # own tiles
NT2 = 16          # 256-wide fp8 pair tiles (global)
NSLAB = 32        # W^T n-slabs of 128 rows
QPAD = 32         # class-dim padding for fp8 DoubleRow lhsT stride
CHUNK = NTO // 2  # own tiles per exchange chunk
NITERS = int(os.environ.get("CRF_NITERS", "5"))
NCORES = 8
RG = [[0, 1], [2, 3], [4, 5], [6, 7]]

LAST_EXEC_NS = None
_CACHE = {}


def _install_ntff_hook():
    """Best-effort registration of the axon NTFF profile hook (image antenv
    lacks axon_hooks, so trn_boot could not register it)."""
    try:
        import types

        if "antenv.axon_hooks" in sys.modules:
            return
        holder = [None]
        m = types.ModuleType("antenv.axon_hooks")
        m.set_axon_ntff_profile_hook = lambda h: holder.__setitem__(0, h)
        m.get_axon_ntff_profile_hook = lambda: holder[0]
        sys.modules["antenv.axon_hooks"] = m
        import antenv

        antenv.axon_hooks = m
        from trn_agent_boot.trn_boot import _ntff_profile_via_ctypes

        m.set_axon_ntff_profile_hook(
            _ntff_profile_via_ctypes("/opt/axon/libaxon_pjrt.so")
        )
    except Exception:
        pass


def _build(niters):
    from concourse import bacc, bass, tile, mybir

    fp32, fp16 = mybir.dt.float32, mybir.dt.float16
    sdt = mybir.dt.float8e4
    AF = mybir.ActivationFunctionType
    ALU = mybir.AluOpType
    ntile = NT2
    half = ntile // 2
    perf = mybir.MatmulPerfMode.DoubleRow

    nc = bacc.Bacc(None, target_bir_lowering=False)

    wt_in_hbm = nc.dram_tensor("w", (N, NH), fp32, kind="ExternalInput")
    segt_in = nc.dram_tensor("segt", (128, NT, D), fp32, kind="ExternalInput")
    wm_in = nc.dram_tensor("wt", (DE, DE), fp32, kind="ExternalInput")
    sel_in = nc.dram_tensor("sel", (128, 2), fp32, kind="ExternalInput")
    out_t = nc.dram_tensor("out", (128, NTO, D), fp32, kind="ExternalOutput")

    n_ex = max(0, niters - 1)
    cc_ins = [
        [
            nc.dram_tensor(f"cc_in{k}_{c}", (128, CHUNK * QPAD), sdt, kind="Internal")
            for c in range(2)
        ]
        for k in range(n_ex)
    ]
    cc_outs = [
        [
            nc.dram_tensor(
                f"cc_out{k}_{c}", (2, 128, CHUNK * QPAD), sdt, kind="Internal"
            )
            for c in range(2)
        ]
        for k in range(n_ex)
    ]

    with tile.TileContext(nc) as tc:
        with (
            tc.tile_pool(name="wt_res", bufs=1) as wt_res,
            tc.tile_pool(name="slab32", bufs=4) as slab32p,
            tc.tile_pool(name="state", bufs=1) as state,
            tc.tile_pool(name="qt", bufs=2) as qtp,
            tc.tile_pool(name="work", bufs=2) as work,
            tc.tile_pool(name="ps_mm", bufs=1, space=bass.MemorySpace.PSUM) as ps_mm,
            tc.tile_pool(name="ps_ut", bufs=2, space=bass.MemorySpace.PSUM) as ps_ut,
            tc.tile_pool(name="ps_jk", bufs=1, space=bass.MemorySpace.PSUM) as ps_jk,
        ):
            # ---- small inputs (ACT HWDGE ring; slab DMAs own the SP ring) --
            wm32 = state.tile([DE, DE], fp32)
            nc.scalar.dma_start(wm32[:], wm_in[:])
            wm16 = state.tile([DE, DE], fp16)
            nc.gpsimd.tensor_copy(wm16[:], wm32[:])
            segt = state.tile([128, NT, D], fp32)
            nc.scalar.dma_start(segt[:], segt_in[:])
            selt = state.tile([128, 2], fp32)
            nc.scalar.dma_start(selt[:], sel_in[:])
            zbias = state.tile([128, 1], fp32)
            nc.gpsimd.memset(zbias[:], 0.0)
            # mask for predicated partner select: nonzero where slot1=partner
            selmask = state.tile([128, CHUNK * QPAD], mybir.dt.uint8)
            nc.gpsimd.tensor_scalar_mul(
                selmask[:],
                selt[:, 1:2].broadcast_to((128, CHUNK * QPAD)),
                1.0,
            )

            # ---- initial Q = softmax(uniqs); ones in column D for rowsum ---
            ex0 = state.tile([128, NT, D], fp32)
            nc.scalar.activation(ex0[:], segt[:], AF.Exp, bias=zbias[:])
            ssum0 = state.tile([128, NT], fp32)
            nc.vector.reduce_sum(ssum0[:], ex0[:], axis=mybir.AxisListType.X)
            srecip0 = state.tile([128, NT], fp32)
            nc.vector.reciprocal(srecip0[:], ssum0[:])
            qt_own = qtp.tile([128, NTO, QPAD], sdt, tag="qt_own", name="qt_own0")
            qt_par = qtp.tile([128, NTO, QPAD], sdt, tag="qt_par", name="qt_par0")
            nc.vector.tensor_tensor(
                qt_own[:, :, 0:D],
                ex0[:, 0:NTO, :],
                srecip0[:, 0:NTO, None].broadcast_to((128, NTO, D)),
                ALU.mult,
            )
            nc.vector.tensor_tensor(
                qt_par[:, :, 0:D],
                ex0[:, NTO:NT, :],
                srecip0[:, NTO:NT, None].broadcast_to((128, NTO, D)),
                ALU.mult,
            )
            nc.gpsimd.memset(qt_own[:, :, D : D + 1], 1.0)
            nc.gpsimd.memset(qt_par[:, :, D : D + 1], 1.0)

            # ---- resident W^T (fp8, pair-interleaved for DoubleRow) ------
            # wt_all[p, t2, i, j] = W^T[256*t2 + 128*i + p, j]
            wt_all = state.tile([128, NT2, 2, NH], sdt, name="wt_all")
            rs_recg = [
                state.tile([128, 4], fp32, tag=f"rsrec{g}", name=f"rs_rec{g}")
                for g in range(4)
            ]

            # narrow filler matmuls: keep the PE instruction stream dense
            # through cross-engine waits so the HAM clock gate stays open
            junk = ps_jk.tile([D, 512], fp32, name="junk")

            def fillers(n):
                for f in range(n):
                    nc.tensor.matmul(
                        junk[:, 0:D],
                        wm16[0:D, 0:D],
                        wm16[0:D, 0:D],
                        start=True,
                        stop=True,
                    )

            def lhs_of(t, q_own, q_par, de):
                src = q_own if t < half else q_par
                j2 = t % half
                return src[:, 2 * j2 : 2 * j2 + 2, 0:de]

            def rhs_of(t, mc):
                return wt_all[:, t, :, mc * 512 : (mc + 1) * 512]

            class IterEmitter:
                """Emits one mean-field iteration in dependency-friendly
                pieces so matmuls, evacuations, and the softmax tail
                pipeline across engines (and, for iteration 0, interleave
                with the prepass)."""

                def __init__(self, it, q_own, q_par, last):
                    self.it, self.q_own, self.q_par, self.last = it, q_own, q_par, last
                    self.de = DE if it == 0 else D
                    self.pP = ps_mm.tile([DE, NH], fp32, tag="pp", name=f"pp{it}")
                    self.ps16g = [None] * 4
                    self.pUTg = [None] * 4
                    self.qt_next = None
                    if not last:
                        self.qt_next = qtp.tile(
                            [128, NTO, QPAD], sdt, tag="qt_own", name=f"qt_own{it+1}"
                        )

                def phase(self, mms):
                    for t, mc in mms:
                        nc.tensor.matmul(
                            self.pP[0 : self.de, mc * 512 : (mc + 1) * 512],
                            lhs_of(t, self.q_own, self.q_par, self.de),
                            rhs_of(t, mc),
                            start=(t == 0),
                            stop=(t == ntile - 1),
                            perf_mode=perf,
                        )

                def evac(self, mc, on_act=None):
                    t16 = work.tile(
                        [DE, 512], fp16, tag=f"ps16_{mc}", name=f"ps16_{self.it}_{mc}"
                    )
                    src = self.pP[0 : self.de, mc * 512 : (mc + 1) * 512]
                    if on_act is None:
                        on_act = mc % 2 == 1
                    if on_act:
                        nc.scalar.activation(t16[0 : self.de, :], src, AF.Copy)
                    else:
                        nc.vector.tensor_copy(t16[0 : self.de, :], src)
                    self.ps16g[mc] = t16

                def ut(self, g):
                    de = self.de
                    pu = ps_ut.tile(
                        [128, 4, DE], fp32, tag="put", name=f"pUT{self.it}_{g}"
                    )
                    for jj in range(4):
                        nc.tensor.matmul(
                            pu[:, jj, 0:de],
                            self.ps16g[g][0:de, jj * 128 : (jj + 1) * 128],
                            wm16[0:de, 0:de],
                            start=True,
                            stop=True,
                        )
                    self.pUTg[g] = pu
                    if self.it == 0:
                        # PSUM row D carried the W row-sums through ut
                        nc.vector.reciprocal(rs_recg[g][:], pu[:, :, D])

                def tail(self, g):
                    it, sl = self.it, slice(4 * g, 4 * g + 4)
                    upd = work.tile([128, 4, D], fp32, tag=f"upd{g}", name=f"upd{it}_{g}")
                    nc.vector.tensor_tensor(
                        upd[:],
                        self.pUTg[g][:, :, 0:D],
                        rs_recg[g][:, :, None].broadcast_to((128, 4, D)),
                        ALU.mult,
                    )
                    qv = work.tile([128, 4, D], fp32, tag=f"qv{g}", name=f"qv{it}_{g}")
                    nc.gpsimd.tensor_tensor(qv[:], segt[:, sl, :], upd[:], ALU.subtract)
                    if self.last:
                        nc.sync.dma_start(out_t[:, sl, :], qv[:])
                        return
                    exq = work.tile([128, 4, D], fp32, tag=f"exq{g}", name=f"exq{it}_{g}")
                    nc.scalar.activation(exq[:], qv[:], AF.Exp, bias=zbias[:])
                    ssum = work.tile([128, 4], fp32, tag=f"ssum{g}", name=f"ssum{it}_{g}")
                    nc.vector.reduce_sum(ssum[:], exq[:], axis=mybir.AxisListType.X)
                    srec = work.tile([128, 4], fp32, tag=f"srec{g}", name=f"srec{it}_{g}")
                    nc.vector.reciprocal(srec[:], ssum[:])
                    nc.vector.tensor_tensor(
                        self.qt_next[:, sl, 0:D],
                        exq[:],
                        srec[:, :, None].broadcast_to((128, 4, D)),
                        ALU.mult,
                    )

                def send_chunk(self, c):
                    """Send own tiles [8c, 8c+8) of qt_next to the partner.
                    The recv side is emitted by the NEXT iteration (recv_chunk)
                    so no recv DMA ever queues ahead of a later send on the
                    in-order sync ring (a recv blocks on AllGather completion)."""
                    it = self.it
                    csl = slice(CHUNK * c, CHUNK * (c + 1))
                    nc.sync.dma_start(
                        cc_ins[it][c][:],
                        self.qt_next[:, csl, :].rearrange("p a b -> p (a b)"),
                    )
                    nc.gpsimd.collective_compute(
                        "AllGather",
                        ALU.bypass,
                        replica_groups=RG,
                        ins=[cc_ins[it][c][:].opt()],
                        outs=[cc_outs[it][c][:].opt()],
                    )

                def recv_chunk(self, c):
                    """Land the partner half of exchange chunk c of the
                    PREVIOUS round into this iteration's qt_par."""
                    ex = self.it - 1
                    csl = slice(CHUNK * c, CHUNK * (c + 1))
                    dst = self.q_par[:, csl, :].rearrange("p a b -> p (a b)")
                    g1 = work.tile(
                        [128, CHUNK * QPAD], sdt, tag="g1", name=f"g1_{ex}_{c}"
                    )
                    nc.sync.dma_start(dst, cc_outs[ex][c][0][:])
                    nc.sync.dma_start(g1[:], cc_outs[ex][c][1][:])
                    nc.vector.copy_predicated(dst, selmask[:], g1[:])

            # ---- prepass: stream W^T slabs, cast to fp8, fold in iter 0 ----
            fillers(12)
            em = IterEmitter(0, qt_own, qt_par, last=(niters == 1))
            for s in range(NSLAB):
                t2, i = s // 2, s % 2
                w32 = slab32p.tile([128, NH], fp32, tag="w32", name=f"w32_{s}")
                nc.sync.dma_start(w32[:], wt_in_hbm[s * 128 : (s + 1) * 128, :])
                dst = wt_all[:, t2, i, :]
                if s % 8 in (2, 5, 7):
                    nc.vector.tensor_copy(dst, w32[:])
                else:
                    nc.scalar.activation(dst, w32[:], AF.Copy)
                if i == 1:
                    em.phase([(t2, mc) for mc in range(4)])
                    fillers(14)
            for g in range(4):
                em.evac(g, on_act=(g % 2 == 1))
                em.ut(g)
                em.tail(g)
                if niters > 1:
                    if g == 1:
                        em.send_chunk(0)
                    elif g == 3:
                        em.send_chunk(1)
            if niters > 1:
                qt_own = em.qt_next

            # ---- iterations 1..niters-1 ---------------------------------
            for it in range(1, niters):
                qt_par = qtp.tile(
                    [128, NTO, QPAD], sdt, tag="qt_par", name=f"qt_par{it}"
                )
                em = IterEmitter(it, qt_own, qt_par, last=(it == niters - 1))
                em.recv_chunk(0)
                # own tiles, gated per 4-tile group on the producing tail;
                # mc=3 last (its pP chunk is freed latest by the prev iter)
                for g in range(4):
                    em.phase([(t, mc) for t in (2 * g, 2 * g + 1) for mc in range(3)])
                em.phase([(t, 3) for t in range(half)])
                em.recv_chunk(1)
                if it == 1:
                    fillers(24)
                # partner tiles, first exchange chunk
                em.phase([(t, mc) for t in range(half, half + 4) for mc in range(4)])
                # partner tiles, second chunk; per-mc finish -> evac/ut/tail
                for mc in range(4):
                    em.phase([(t, mc) for t in range(half + 4, ntile)])
                    em.evac(mc, on_act=(mc in (1, 2)))
                    em.ut(mc)
                    em.tail(mc)
                    if it < niters - 1:
                        if mc == 1:
                            em.send_chunk(0)
                        elif mc == 3:
                            em.send_chunk(1)
                if it < niters - 1:
                    qt_own = em.qt_next

    nc.compile()
    return nc


def _get_nc(niters):
    if niters not in _CACHE:
        _CACHE[niters] = _build(niters)
    return _CACHE[niters]


def kernel(seg, W, weights):
    global LAST_EXEC_NS
    assert seg.shape == (BS, D, RC, RC) and W.shape == (BS, N, N)
    trace = bool(os.environ.get("BASS_TRACE"))
    if trace:
        _install_ntff_hook()

    from concourse.bass_utils import run_bass_kernel_spmd

    nc = _get_nc(NITERS)

    seg32 = np.ascontiguousarray(seg, dtype=np.float32)
    W32 = np.ascontiguousarray(W, dtype=np.float32)
    # weights^T padded to 22x22 with a unit diagonal tail: through the `ut`
    # transpose-matmul, ps16 row 21 (the accumulated W row-sums) passes into
    # pUT column 21 untouched while columns 0..20 get weights @ seg_diff.
    wm_np = np.zeros((DE, DE), np.float32)
    wm_np[0:D, 0:D] = weights.T
    wm_np[D, D] = 1.0

    in_maps = []
    for c in range(NCORES):
        b, h = c // 2, c % 2
        own = slice(NH * h, NH * h + NH)
        par = slice(NH * (1 - h), NH * (1 - h) + NH)
        Wb = W32[b]
        # host-side layout prep: the device wants W^T (contraction index n on
        # partitions), rows in [own, partner] order to match the Q tiles
        w_np = np.ascontiguousarray(
            np.concatenate([Wb[own, own], Wb[own, par]], axis=1).T
        )
        st = seg32[b].reshape(D, N).T  # [n, d]
        st_perm = np.concatenate([st[own], st[par]], axis=0)
        segt_np = np.ascontiguousarray(
            st_perm.reshape(NT, 128, D).transpose(1, 0, 2)
        )
        sel_np = np.zeros((128, 2), np.float32)
        sel_np[:, 0] = float(h)       # gather slot (1-h) = partner
        sel_np[:, 1] = float(1 - h)
        in_maps.append(
            {"w": w_np, "segt": segt_np, "wt": wm_np, "sel": sel_np}
        )

    res = run_bass_kernel_spmd(
        nc, in_maps, core_ids=list(range(NCORES)), trace=trace
    )
    LAST_EXEC_NS = res.exec_time_ns

    out = np.empty((BS, D, N), np.float32)
    for c in range(NCORES):
        b, h = c // 2, c % 2
        qv = res.results[c]["out"]  # [128, NTO, D]
        block = qv.transpose(2, 1, 0).reshape(D, NH)
        out[b][:, NH * h : NH * h + NH] = block
    return out.reshape(BS, D, RC, RC)


if __name__ == "__main__":
    rng = np.random.default_rng(0)
    seg = rng.standard_normal((BS, D, RC, RC)).astype(np.float32)
    W = rng.random((BS, N, N), dtype=np.float32)
    weights = rng.standard_normal((D, D)).astype(np.float32)
    out = kernel(seg=seg, W=W, weights=weights)
    print("out", out.shape, out.dtype, float(np.abs(out).mean()))
